# revision 36
# baseline (speedup 1.0000x reference)
"""BatchHardTripletLoss (with faithful source bug) on 8 Trainium2 NeuronCores.

Reference semantics (N=8192, D=128, C=10 classes, margin=1.0):
    d(i,j)   = max(x2_i + x2_j - 2 e_i.e_j, 0)
    d_pos[i] = max_{j: same class} d(i,j)                  (includes self)
    S[i,k]   = sum_{j: class k} d(i,j);  k* = argmax_k S[i,k]
    j*       = (k*)-th negative of i in (class, index) order
    loss     = mean relu(d_pos - d(i,j*) + 1)

Same mathematical structure as the v1 kernel (class-sorted padded blocks,
closed-form S, candidate columns), rebuilt around what the v1 trace showed:

  * v1 spent ~7us loading inputs before the first matmul.  v2 cuts the DMA
    footprint ~2.2x: the block-mb window IS the anchor tensor (both sides
    are stored as sqrt2*e, the DVE op computes x2_j - psum instead of
    psum + x2_j), and the [128, 2Wr] broadcast x2_j tensor is replaced by a
    [1, 2Wr] row expanded on-chip by a K=1 ones-matmul + scalar-engine copy.
  * transfers are issued on the two HWDGE queues (sync/scalar) in exactly
    the order the loop consumes them; only the late-needed leftover-block
    window rides the slow gpsimd SWDGE queue.
  * the per-tile [128,20] aux results stay resident in one PSUM bank for the
    whole loop (no psum->sbuf copies); the mining epilogue reads PSUM
    directly and is batched into ~8 whole-[128,Q*10] DVE ops instead of
    10 tiny serialized scalar_tensor_tensor calls.
  * junk matmuls at t=0 keep the PE busy so its DVFS ramp (0.65 -> 2.4GHz
    after ~3us of continuous work) has progressed before the real window
    matmuls arrive.

Device layout: rows and columns are class-sorted; every class block is padded
to a uniform width B (pad = duplicate of the block's first member).  One NEFF
with static shapes serves all 8 cores; per-core variation is data-only.
Each core gets Q = 10*B/128/8 anchor tiles: one whole "main" block plus a
slice of one leftover block.

Host does only O(N*D) input marshalling (sort/pad/stats); all O(N*B*D) work
plus the mining runs on the NeuronCores.
"""

import numpy as np
from contextlib import ExitStack

import ml_dtypes
import concourse.bass as bass
import concourse.tile as tile
from concourse import bacc, mybir
from concourse import dve_ops
from concourse.dve_spec import Spec, Src0, Src1, C0, minn, lower, _has_src1
from concourse.dve_uop import DveOpSpec
from concourse.bass_utils import run_bass_kernel_spmd

N_CORES = 8
C = 10
MARGIN = 1.0
P = 128
F32 = mybir.dt.float32
BF16 = mybir.dt.bfloat16
AX = mybir.AxisListType.X
ALU = mybir.AluOpType
POS_INF = 3.0e38
PAD_NEG = -1.0e30
SQ2 = float(np.sqrt(2.0))

# stash of the last BassKernelResults (read by test.py for profiling)
last_results = None
_trace_opts: dict = {}


def _ref_add_min_reduce(in0, in1, c0, c1, c2):
    b = (np.asarray(in0, np.float32) + np.asarray(in1, np.float32))
    if isinstance(c0, np.ndarray):
        seed = np.asarray(c0, np.float32).reshape(-1, 1)
    else:
        seed = np.full((b.shape[0], 1), float(c0), np.float32)
    acc = np.minimum(seed, b.reshape(b.shape[0], -1).min(axis=-1, keepdims=True))
    return b.astype(np.float32), acc.astype(np.float32)


def _register_add_min_reduce():
    """Custom DVE op: out = in0 + in1; accum_out = min(s0, rowmin(out)).

    in0 is the PSUM gram tile (+2 e_i.e_j from the sqrt2-scaled anchors),
    in1 the broadcast NEGATED x2_j row, so accum = min_j(2 e_i.e_j - x2_j)
    = -max_j(x2_j - 2 e_i.e_j) in one DVE pass over the PSUM tile.  (ADD
    with a min-accumulator: the v2 trace showed Src1-Src0 lowers to a
    slower uop chain than the commutative ADD.)"""
    name = "ADD_MIN_REDUCE_BHTL"
    for op in dve_ops.OPS:
        if op.name == name:
            return op
    spec = Spec(body=Src0 + Src1, accum=minn, accum_init=C0,
                reference=_ref_add_min_reduce)
    row = dve_ops._CUSTOM_DVE_ROW_BASE + len(dve_ops.OPS)
    assert row < 0x20
    dve_ops._SUB_OPCODE_FOR_NAME[name] = row
    shas = {}
    for ver in ("v3", "v4"):
        try:
            u = lower(spec, ver=ver)
            shas[ver] = DveOpSpec(name=name, opcode=row, uops=u,
                                  rd1_en=_has_src1(spec)).sha(ver)
        except Exception:
            pass
    assert shas, "ADD_MIN_REDUCE_BHTL failed to lower for any DVE version"
    op = dve_ops.DveOp(name, spec, subdim=False, uops_sha=shas)
    dve_ops.OPS.append(op)
    dve_ops.CUSTOM_DVE_SPECS[name] = spec
    return op


ADD_MIN_REDUCE = _register_add_min_reduce()


def _build_program(B: int, Q: int, Wr: int):
    """One SPMD program; all per-core variation is in the input tensors.

    B: padded class-block width (1024), Q: anchor tiles per core, Wr: window
    columns actually read (global max class count).
    """
    TB = B // P            # tiles in the main block
    W1 = min(512, Wr)      # window chunk widths
    W2 = Wr - W1
    WB = TB * P - 512      # aB width (main-block cols past 512)
    nc = bacc.Bacc("TRN2", target_bir_lowering=False, debug=False,
                   num_devices=N_CORES)

    aA_d = nc.dram_tensor("aA", [P, 512], BF16, kind="ExternalInput").ap()
    aB_d = nc.dram_tensor("aB", [P, WB], BF16, kind="ExternalInput").ap()
    aC_d = nc.dram_tensor("aC", [P, (Q - TB) * P], BF16,
                          kind="ExternalInput").ap()
    web_d = nc.dram_tensor("web", [P, Wr], BF16, kind="ExternalInput").ap()
    x2r_d = nc.dram_tensor("x2r", [1, 2 * Wr], BF16, kind="ExternalInput").ap()
    sc_d = nc.dram_tensor("sc", [P, Q * 20], BF16, kind="ExternalInput").ap()
    sc2_d = nc.dram_tensor("sc2", [4, Q * 20], BF16, kind="ExternalInput").ap()
    a2_d = nc.dram_tensor("a2", [4, Q * P], BF16, kind="ExternalInput").ap()
    xm_d = nc.dram_tensor("x2a1", [P, Q], F32, kind="ExternalInput").ap()
    out_d = nc.dram_tensor("out", [1, 1], F32, kind="ExternalOutput").ap()

    with tile.TileContext(nc) as tc, ExitStack() as ctx:
        const = ctx.enter_context(tc.tile_pool(name="const", bufs=1))
        pwin = ctx.enter_context(tc.tile_pool(name="pwin", bufs=3,
                                              space="PSUM"))
        paux = ctx.enter_context(tc.tile_pool(name="paux", bufs=1,
                                              space="PSUM"))
        scratch = ctx.enter_context(tc.tile_pool(name="scratch", bufs=2))

        # --- constants (gpsimd memsets come first: gpsimd later emits the
        # SWDGE descriptors for several inputs) ---
        onesb = const.tile([1, P], BF16)
        nc.gpsimd.memset(onesb[:], 1.0)
        ones_f = const.tile([P, 1], F32)
        nc.gpsimd.memset(ones_f[:], 1.0)

        # --- input SBUF tiles + DMA issue, in consumption order ---
        aA = const.tile([P, 512], BF16)
        aB = const.tile([P, WB], BF16)
        aC = const.tile([P, (Q - TB) * P], BF16)
        web = const.tile([P, Wr], BF16)
        x2r = const.tile([1, 2 * Wr], BF16)
        sc_sb = const.tile([P, Q * 20], BF16)
        sc2_sb = const.tile([4, Q * 20], BF16)
        a2_sb = const.tile([4, Q * P], BF16)
        xm_sb = const.tile([P, Q], F32)

        # x2r rides first on the sync HWDGE queue: the x2 broadcast is the
        # head of the tensor stream.  scalar does only aB then is free for
        # the ACT broadcast copies; gpsimd (SWDGE) carries the late-needed
        # rest.  The descriptor-generation instructions are pinned at
        # priority 0: pinning only the compute stream demotes them in the
        # schedule and the whole pipeline slips ~0.5us.
        with tc.high_priority():
            nc.sync.dma_start(x2r[:], x2r_d[:])
            nc.sync.dma_start(aA[:], aA_d[:])
            nc.scalar.dma_start(aB[:], aB_d[:])
            nc.sync.dma_start(aC[:], aC_d[:])
            nc.gpsimd.dma_start(sc_sb[:], sc_d[:])
            nc.gpsimd.dma_start(a2_sb[:], a2_d[:])
            nc.gpsimd.dma_start(sc2_sb[:], sc2_d[:])
            nc.gpsimd.dma_start(web[:], web_d[:])
            nc.gpsimd.dma_start(xm_sb[:], xm_d[:])

        # --- x2 row -> [128, Wr] per block, via K=1 ones-matmul + ACT copy
        # (negated x2 is loaded, so the mining op is ADD + min-accum);
        # per-chunk ACT copies overlap the next chunk's matmul
        x2jp = [const.tile([P, Wr], BF16, name=f"x2jp{b}") for b in range(2)]

        def emit_x2_bcast(blk):
            # one pwin tile per block, both chunk matmuls, ONE ACT copy after
            # (a copy between the two matmuls would serialize chunk-2 behind
            # it via tile-granular WAR tracking); rides the window pool so
            # all 8 PSUM banks go to pwin(3x2)+paux, keeping 3 windows in
            # flight -- with only 2, the scheduler saw win2 blocked and
            # packed the whole aux blob ahead of it (3.4us DVE stall in v6)
            px = pwin.tile([P, Wr], F32, tag="ps", name=f"x2b{blk}")
            for (c0, cw) in ((0, W1), (W1, W2)):
                nc.tensor.matmul(px[:, c0:c0 + cw], onesb[:],
                                 x2r[0:1, blk * Wr + c0:blk * Wr + c0 + cw],
                                 start=True, stop=True)
            nc.scalar.copy(x2jp[blk][:], px[:])

        # block-0 broadcast feeds the first mining op: pin it to the head of
        # the tensor/scalar queues (the scheduler otherwise floats the second
        # chunk behind the whole aux batch, stalling the DVE pipeline ~4us)
        with tc.high_priority():
            emit_x2_bcast(0)

        mall = const.tile([P, Q], F32)    # min_j(2 e_i.e_j - x2_j)
        pv = paux.tile([P, Q * 20 + 24], F32)  # [S | d_cand] + pout column

        def win_lhs(t):
            if t < 4:
                return aA[:, t * P:(t + 1) * P]
            if t < TB:
                return aB[:, (t - 4) * P:(t - 3) * P]
            return aC[:, (t - TB) * P:(t - TB + 1) * P]

        def emit_aux(t):
            # deferred one tile behind the windows: results are only read by
            # the epilogue.  (Per-tile tile_wait_until staggering was tried
            # and REGRESSED: the model-time hints insert real scheduling
            # bubbles; the early aux blob costs less than the bubbles.)
            scol = slice(t * 20, (t + 1) * 20)
            nc.tensor.matmul(pv[:, scol], win_lhs(t), sc_sb[:, scol],
                             start=True, stop=False)
            nc.tensor.matmul(pv[:, scol], a2_sb[:, t * P:(t + 1) * P],
                             sc2_sb[:, scol], start=False, stop=True)

        for t in range(Q):
            blk = 0 if t < TB else 1
            lhs = win_lhs(t)
            rhs1 = aA[:, 0:W1] if blk == 0 else web[:, 0:W1]
            rhs2 = aB[:, 0:W2] if blk == 0 else web[:, W1:Wr]

            # the whole critical chain (descriptors above, x2 broadcasts,
            # windows) is pinned at priority 0 in consumption order; the aux
            # pairs fill the remaining tensor slots.  Without this the
            # scheduler models the custom DVE ops as slow, concludes the
            # windows are psum-blocked, and packs the whole aux blob ahead
            # of them (3.9us DVE stall).
            ps = pwin.tile([P, Wr], F32, tag="ps", name=f"ps{t}")
            with tc.high_priority():
                nc.tensor.matmul(ps[:, 0:W1], lhs, rhs1, start=True,
                                 stop=True)
                nc.tensor.matmul(ps[:, W1:Wr], lhs, rhs2, start=True,
                                 stop=True)

            dsc = scratch.tile([P, Wr], F32, tag="dsc")
            nc.vector._custom_dve(ADD_MIN_REDUCE, out=dsc[:],
                                  in0=ps[:], in1=x2jp[blk][:],
                                  s0=POS_INF, accum_out=mall[:, t:t + 1])

            if t == 3:
                with tc.high_priority():
                    emit_x2_bcast(1)  # block-1 x2, needed from tile TB
            if t >= 1:
                emit_aux(t - 1)
        emit_aux(Q - 1)

        # ---- batched mining epilogue (reads the aux PSUM directly) ----
        # tile_wait_until pushes these into the scheduler's far future so the
        # in-order DVE queue keeps the mining ops (emitted above) first; the
        # v2 trace showed the scheduler hoisting this chain ahead of them.
        sv3 = pv[:, 0:Q * 20].rearrange("p (q s) -> p q s", s=20)
        smax = const.tile([P, Q], F32)
        mask = const.tile([P, Q * 10], F32)
        mask3 = mask[:].rearrange("p (q s) -> p q s", s=10)
        prod = const.tile([P, Q * 10], F32)
        prod3 = prod[:].rearrange("p (q s) -> p q s", s=10)
        dneg = const.tile([P, Q], F32)
        # the dneg subchain only needs the aux results (all drained early by
        # the scheduler) -- hint it into the mid-loop DVE bubbles left by the
        # window stalls; worst case it displaces mining ops one-for-one
        with tc.tile_wait_until(0.0078):
            nc.vector.reduce_max(smax[:], sv3[:, :, 0:10], axis=AX)
            smax_b = smax[:].rearrange("p (q one) -> p q one", one=1) \
                .to_broadcast((P, Q, 10))
            nc.vector.tensor_tensor(mask3, sv3[:, :, 0:10], smax_b,
                                    op=ALU.is_equal)
            nc.vector.tensor_tensor(prod3, mask3, sv3[:, :, 10:20],
                                    op=ALU.mult)
            nc.vector.reduce_sum(dneg[:], prod3, axis=AX)

        with tc.tile_wait_until(0.030):
            # xm already carries +margin (folded host-side, exact in f32).
            # NOTE: tensor_scalar with accum_out (TensorScalarPtrReduce)
            # hard-faults this runtime (NRT_EXEC_UNIT_UNRECOVERABLE) — keep
            # the relu and the row-sum as separate instructions.
            t1 = const.tile([P, Q], F32)
            nc.vector.tensor_sub(t1[:], xm_sb[:], mall[:])  # x2_i+m - min
            t2 = const.tile([P, Q], F32)
            nc.vector.tensor_sub(t2[:], t1[:], dneg[:])
            t3 = const.tile([P, Q], F32)
            nc.vector.tensor_scalar(t3[:], t2[:], 0.0, None, op0=ALU.max)
            lsum = const.tile([P, 1], F32)
            nc.vector.reduce_sum(lsum[:], t3[:], axis=AX)
            # partition-sum via a 1-column matmul so the output DMA is a
            # single 4-byte transfer
            nc.tensor.matmul(pv[0:1, Q * 20 + 20:Q * 20 + 21], lsum[:],
                             ones_f[:], start=True, stop=True)
            res_sb = const.tile([1, 1], F32)
            nc.vector.tensor_copy(res_sb[:], pv[0:1, Q * 20 + 20:Q * 20 + 21])
            nc.sync.dma_start(out_d[:], res_sb[:])

    nc.compile()
    return nc


_prog_cache: dict = {}


def kernel(embeddings: np.ndarray, labels: np.ndarray) -> np.ndarray:
    global last_results
    e = np.ascontiguousarray(np.asarray(embeddings), dtype=np.float32)
    lab = np.asarray(labels).astype(np.int64)
    N, D = e.shape
    assert D == P and N % N_CORES == 0

    # ---- host-side marshalling: class-sort, pad, per-class stats ----
    order = np.argsort(lab * N + np.arange(N))
    e = e[order]
    lab_s = lab[order]
    cnt = np.bincount(lab_s, minlength=C)
    assert len(cnt) == C and cnt[0] >= 10 and cnt[1] >= 10, cnt
    offs = np.zeros(C + 1, dtype=np.int64)
    offs[1:] = np.cumsum(cnt)

    # block width: per-class tile count, uniform across classes (all counts
    # land in the same 128-bucket for this regime).  C*TB tiles split over
    # the cores; slots past the real tiles are loss-squashed dummies.
    B = int(-(-int(cnt.max()) // P) * P)
    TB = B // P
    Q = -(-C * TB // N_CORES)
    L = Q - TB            # leftover slots per core
    n_left = C - N_CORES  # leftover blocks (2)
    cpl = N_CORES // n_left  # cores sharing one leftover block
    assert n_left == 2 and L * cpl >= TB

    x2 = np.einsum("nd,nd->n", e, e).astype(np.float32)
    NP_ = C * B
    ep = np.empty((NP_, D), np.float32)
    x2p = np.empty(NP_, np.float32)
    validp = np.zeros(NP_, np.float32)
    for k in range(C):
        m = int(cnt[k])
        blk = e[offs[k]:offs[k + 1]]
        ep[k * B:k * B + m] = blk
        ep[k * B + m:(k + 1) * B] = blk[0]
        x2p[k * B:k * B + m] = x2[offs[k]:offs[k + 1]]
        x2p[k * B + m:(k + 1) * B] = x2[offs[k]]
        validp[k * B:k * B + m] = 1.0
    # bf16-rounded x2_i, shared by the fp32 aux matmul and x2a1 so the x2_i
    # term cancels exactly in d_pos - d_neg
    x2p_bf32 = x2p.astype(ml_dtypes.bfloat16).astype(np.float32)

    E = np.stack([e[offs[k]:offs[k + 1]].sum(axis=0) for k in range(C)],
                 axis=1).astype(np.float32)          # [D, C]
    Ck = np.array([x2[offs[k]:offs[k + 1]].sum() for k in range(C)],
                  dtype=np.float32)                  # [C]
    candA = e[0:10]                                  # class-0 members
    candB = e[offs[1]:offs[1] + 10]                  # class-1 members
    x2A, x2B = x2[0:10], x2[offs[1]:offs[1] + 10]

    Wr = int(cnt.max())
    key = (B, Q, Wr)
    if key not in _prog_cache:
        _prog_cache[key] = _build_program(B, Q, Wr)
    nc = _prog_cache[key]

    in_maps = []
    for c in range(N_CORES):
        mb = c                      # main block
        eb = N_CORES + c // cpl     # leftover block index
        base_et = (c % cpl) * L     # first leftover tile within it
        row_segs = [np.arange(mb * B, (mb + 1) * B)]
        dummy_slot = []
        for j in range(L):
            et = base_et + j
            if et >= TB:            # dummy slot: replicate tile 0, squash
                et = 0
                dummy_slot.append(TB + j)
            row_segs.append(np.arange(eb * B + et * P, eb * B + (et + 1) * P))
        rows = np.concatenate(row_segs)
        tile_cls = [mb] * TB + [eb] * L

        anchT = ep[rows].T                          # [D, Q*128]
        a = (SQ2 * anchT).astype(ml_dtypes.bfloat16)
        web = (SQ2 * ep[eb * B:eb * B + Wr].T).astype(ml_dtypes.bfloat16)
        x2r = -np.concatenate([x2p[mb * B:mb * B + Wr],
                               x2p[eb * B:eb * B + Wr]])[None, :] \
            .astype(ml_dtypes.bfloat16)
        # aux lhsT rows [x2_i; 1; x2_i; 1] pair with hi/lo-split rhs rows so
        # every aux product is bf16-exact (bf16 alone cannot hold cnt_k / C_k)
        a2 = np.stack([x2p_bf32[rows], np.ones(Q * P, np.float32),
                       x2p_bf32[rows], np.ones(Q * P, np.float32)])
        sc = np.empty((D, Q * 20), np.float32)
        sc2 = np.empty((4, Q * 20), np.float32)
        cnt_f = cnt.astype(np.float32)
        cnt_hi = (cnt // 128 * 128).astype(np.float32)
        cnt_lo = cnt_f - cnt_hi
        Ck_hi = Ck.astype(ml_dtypes.bfloat16).astype(np.float32)
        Ck_lo = Ck - Ck_hi
        x2A_hi = x2A.astype(ml_dtypes.bfloat16).astype(np.float32)
        x2B_hi = x2B.astype(ml_dtypes.bfloat16).astype(np.float32)
        for t in range(Q):
            c0 = tile_cls[t] == 0
            cand = candB if c0 else candA
            x2c_hi = x2B_hi if c0 else x2A_hi
            x2c_lo = (x2B - x2B_hi) if c0 else (x2A - x2A_hi)
            sc[:, t * 20:t * 20 + 10] = -SQ2 * E
            sc[:, t * 20 + 10:t * 20 + 20] = -SQ2 * cand.T
            sc2[0, t * 20:t * 20 + 10] = cnt_hi
            sc2[1, t * 20:t * 20 + 10] = Ck_hi
            sc2[2, t * 20:t * 20 + 10] = cnt_lo
            sc2[3, t * 20:t * 20 + 10] = Ck_lo
            sc2[0, t * 20 + 10:t * 20 + 20] = 1.0
            sc2[1, t * 20 + 10:t * 20 + 20] = x2c_hi
            sc2[2, t * 20 + 10:t * 20 + 20] = 0.0
            sc2[3, t * 20 + 10:t * 20 + 20] = x2c_lo
        vmask = validp[rows].reshape(Q, P).T.copy()
        for s in dummy_slot:
            vmask[:, s] = 0.0
        # margin folded in host-side (exact in f32; the bf16-rounded x2 term
        # still cancels against the aux-matmul x2)
        x2a1 = np.where(vmask > 0.5,
                        x2p_bf32[rows].reshape(Q, P).T + MARGIN,
                        PAD_NEG).astype(np.float32).copy()

        in_maps.append({"aA": np.ascontiguousarray(a[:, 0:512]),
                        "aB": np.ascontiguousarray(a[:, 512:TB * P]),
                        "aC": np.ascontiguousarray(a[:, TB * P:Q * P]),
                        "web": web,
                        "x2r": x2r,
                        "sc": sc.astype(ml_dtypes.bfloat16),
                        "sc2": sc2.astype(ml_dtypes.bfloat16),
                        "a2": a2.astype(ml_dtypes.bfloat16),
                        "x2a1": x2a1})

    res = run_bass_kernel_spmd(nc, in_maps, list(range(N_CORES)), **_trace_opts)
    last_results = res
    total = np.float64(0.0)
    for c in range(N_CORES):
        total += res.results[c]["out"].astype(np.float64).sum()
    return np.asarray(total / N, dtype=np.float32)


# revision 37
# speedup vs baseline: 1.2196x; 1.2196x over previous
"""BatchHardTripletLoss (with faithful source bug) on 8 Trainium2 NeuronCores.

Reference semantics (N=8192, D=128, C=10 classes, margin=1.0):
    d(i,j)   = max(x2_i + x2_j - 2 e_i.e_j, 0)
    d_pos[i] = max_{j: same class} d(i,j)                  (includes self)
    S[i,k]   = sum_{j: class k} d(i,j);  k* = argmax_k S[i,k]
    j*       = (k*)-th negative of i in (class, index) order
    loss     = mean relu(d_pos - d(i,j*) + 1)

Same mathematical structure as the v1 kernel (class-sorted padded blocks,
closed-form S, candidate columns), rebuilt around what the v1 trace showed:

  * v1 spent ~7us loading inputs before the first matmul.  v2 cuts the DMA
    footprint ~2.2x: the block-mb window IS the anchor tensor (both sides
    are stored as sqrt2*e, the DVE op computes x2_j - psum instead of
    psum + x2_j), and the [128, 2Wr] broadcast x2_j tensor is replaced by a
    [1, 2Wr] row expanded on-chip by a K=1 ones-matmul + scalar-engine copy.
  * transfers are issued on the two HWDGE queues (sync/scalar) in exactly
    the order the loop consumes them; only the late-needed leftover-block
    window rides the slow gpsimd SWDGE queue.
  * the per-tile [128,20] aux results stay resident in one PSUM bank for the
    whole loop (no psum->sbuf copies); the mining epilogue reads PSUM
    directly and is batched into ~8 whole-[128,Q*10] DVE ops instead of
    10 tiny serialized scalar_tensor_tensor calls.
  * junk matmuls at t=0 keep the PE busy so its DVFS ramp (0.65 -> 2.4GHz
    after ~3us of continuous work) has progressed before the real window
    matmuls arrive.

Device layout: rows and columns are class-sorted; every class block is padded
to a uniform width B (pad = duplicate of the block's first member).  One NEFF
with static shapes serves all 8 cores; per-core variation is data-only.
Each core gets Q = 10*B/128/8 anchor tiles: one whole "main" block plus a
slice of one leftover block.

Host does only O(N*D) input marshalling (sort/pad/stats); all O(N*B*D) work
plus the mining runs on the NeuronCores.
"""

import numpy as np
from contextlib import ExitStack

import ml_dtypes
import concourse.bass as bass
import concourse.tile as tile
from concourse import bacc, mybir
from concourse import dve_ops
from concourse.dve_spec import Spec, Src0, Src1, C0, minn, lower, _has_src1
from concourse.dve_uop import DveOpSpec
from concourse.bass_utils import run_bass_kernel_spmd

N_CORES = 8
C = 10
MARGIN = 1.0
P = 128
F32 = mybir.dt.float32
BF16 = mybir.dt.bfloat16
AX = mybir.AxisListType.X
ALU = mybir.AluOpType
POS_INF = 3.0e38
PAD_NEG = -1.0e30
SQ2 = float(np.sqrt(2.0))

# stash of the last BassKernelResults (read by test.py for profiling)
last_results = None
_trace_opts: dict = {}


def _ref_add_min_reduce(in0, in1, c0, c1, c2):
    b = (np.asarray(in0, np.float32) + np.asarray(in1, np.float32))
    if isinstance(c0, np.ndarray):
        seed = np.asarray(c0, np.float32).reshape(-1, 1)
    else:
        seed = np.full((b.shape[0], 1), float(c0), np.float32)
    acc = np.minimum(seed, b.reshape(b.shape[0], -1).min(axis=-1, keepdims=True))
    return b.astype(np.float32), acc.astype(np.float32)


def _register_add_min_reduce():
    """Custom DVE op: out = in0 + in1; accum_out = min(s0, rowmin(out)).

    in0 is the PSUM gram tile (+2 e_i.e_j from the sqrt2-scaled anchors),
    in1 the broadcast NEGATED x2_j row, so accum = min_j(2 e_i.e_j - x2_j)
    = -max_j(x2_j - 2 e_i.e_j) in one DVE pass over the PSUM tile.  (ADD
    with a min-accumulator: the v2 trace showed Src1-Src0 lowers to a
    slower uop chain than the commutative ADD.)"""
    name = "ADD_MIN_REDUCE_BHTL"
    for op in dve_ops.OPS:
        if op.name == name:
            return op
    spec = Spec(body=Src0 + Src1, accum=minn, accum_init=C0,
                reference=_ref_add_min_reduce)
    row = dve_ops._CUSTOM_DVE_ROW_BASE + len(dve_ops.OPS)
    assert row < 0x20
    dve_ops._SUB_OPCODE_FOR_NAME[name] = row
    shas = {}
    for ver in ("v3", "v4"):
        try:
            u = lower(spec, ver=ver)
            shas[ver] = DveOpSpec(name=name, opcode=row, uops=u,
                                  rd1_en=_has_src1(spec)).sha(ver)
        except Exception:
            pass
    assert shas, "ADD_MIN_REDUCE_BHTL failed to lower for any DVE version"
    op = dve_ops.DveOp(name, spec, subdim=False, uops_sha=shas)
    dve_ops.OPS.append(op)
    dve_ops.CUSTOM_DVE_SPECS[name] = spec
    return op


ADD_MIN_REDUCE = _register_add_min_reduce()


def _build_program(B: int, Q: int, Wr: int):
    """One SPMD program; all per-core variation is in the input tensors.

    B: padded class-block width (1024), Q: anchor tiles per core, Wr: window
    columns actually read (global max class count).
    """
    TB = B // P            # tiles in the main block
    W1 = min(512, Wr)      # window chunk widths
    W2 = Wr - W1
    WB = TB * P - 512      # aB width (main-block cols past 512)
    nc = bacc.Bacc("TRN2", target_bir_lowering=False, debug=False,
                   num_devices=N_CORES)

    aA_d = nc.dram_tensor("aA", [P, 512], BF16, kind="ExternalInput").ap()
    aB_d = nc.dram_tensor("aB", [P, WB], BF16, kind="ExternalInput").ap()
    aC_d = nc.dram_tensor("aC", [P, (Q - TB) * P], BF16,
                          kind="ExternalInput").ap()
    web_d = nc.dram_tensor("web", [P, Wr], BF16, kind="ExternalInput").ap()
    x2r_d = nc.dram_tensor("x2r", [1, 2 * Wr], BF16, kind="ExternalInput").ap()
    sc_d = nc.dram_tensor("sc", [P, Q * 20], BF16, kind="ExternalInput").ap()
    sc2_d = nc.dram_tensor("sc2", [4, Q * 20], BF16, kind="ExternalInput").ap()
    a2_d = nc.dram_tensor("a2", [4, Q * P], BF16, kind="ExternalInput").ap()
    xm_d = nc.dram_tensor("x2a1", [P, Q], F32, kind="ExternalInput").ap()
    out_d = nc.dram_tensor("out", [1, 1], F32, kind="ExternalOutput").ap()

    with tile.TileContext(nc) as tc, ExitStack() as ctx:
        const = ctx.enter_context(tc.tile_pool(name="const", bufs=1))
        pwin = ctx.enter_context(tc.tile_pool(name="pwin", bufs=3,
                                              space="PSUM"))
        paux = ctx.enter_context(tc.tile_pool(name="paux", bufs=1,
                                              space="PSUM"))
        scratch = ctx.enter_context(tc.tile_pool(name="scratch", bufs=2))

        # --- constants (gpsimd memsets come first: gpsimd later emits the
        # SWDGE descriptors for several inputs) ---
        onesb = const.tile([1, P], BF16)
        nc.gpsimd.memset(onesb[:], 1.0)
        ones_f = const.tile([P, 1], F32)
        nc.gpsimd.memset(ones_f[:], 1.0)

        # --- input SBUF tiles + DMA issue, in consumption order ---
        aA = const.tile([P, 512], BF16)
        aB = const.tile([P, WB], BF16)
        aC = const.tile([P, (Q - TB) * P], BF16)
        web = const.tile([P, Wr], BF16)
        x2r = const.tile([1, 2 * Wr], BF16)
        sc_sb = const.tile([P, Q * 20], BF16)
        sc2_sb = const.tile([4, Q * 20], BF16)
        a2_sb = const.tile([4, Q * P], BF16)
        xm_sb = const.tile([P, Q], F32)

        # x2r rides first on the sync HWDGE queue: the x2 broadcast is the
        # head of the tensor stream.  scalar does only aB then is free for
        # the ACT broadcast copies; gpsimd (SWDGE) carries the late-needed
        # rest.  The descriptor-generation instructions are pinned at
        # priority 0: pinning only the compute stream demotes them in the
        # schedule and the whole pipeline slips ~0.5us.
        with tc.high_priority():
            nc.sync.dma_start(x2r[:], x2r_d[:])
            nc.sync.dma_start(aA[:], aA_d[:])
            nc.scalar.dma_start(aB[:], aB_d[:])
            nc.sync.dma_start(aC[:], aC_d[:])
            nc.gpsimd.dma_start(sc_sb[:], sc_d[:])
            nc.gpsimd.dma_start(a2_sb[:], a2_d[:])
            nc.gpsimd.dma_start(sc2_sb[:], sc2_d[:])
            nc.gpsimd.dma_start(web[:], web_d[:])
            nc.gpsimd.dma_start(xm_sb[:], xm_d[:])

        # --- x2 row -> [128, Wr] per block, via K=1 ones-matmul + ACT copy
        # (negated x2 is loaded, so the mining op is ADD + min-accum);
        # per-chunk ACT copies overlap the next chunk's matmul
        x2jp = [const.tile([P, Wr], BF16, name=f"x2jp{b}") for b in range(2)]

        def emit_x2_bcast(blk):
            # one pwin tile per block, both chunk matmuls, ONE ACT copy after
            # (a copy between the two matmuls would serialize chunk-2 behind
            # it via tile-granular WAR tracking); rides the window pool so
            # all 8 PSUM banks go to pwin(3x2)+paux, keeping 3 windows in
            # flight -- with only 2, the scheduler saw win2 blocked and
            # packed the whole aux blob ahead of it (3.4us DVE stall in v6)
            px = pwin.tile([P, Wr], F32, tag="ps", name=f"x2b{blk}")
            for (c0, cw) in ((0, W1), (W1, W2)):
                nc.tensor.matmul(px[:, c0:c0 + cw], onesb[:],
                                 x2r[0:1, blk * Wr + c0:blk * Wr + c0 + cw],
                                 start=True, stop=True)
            nc.scalar.copy(x2jp[blk][:], px[:])

        # block-0 broadcast feeds the first mining op: pin it to the head of
        # the tensor/scalar queues (the scheduler otherwise floats the second
        # chunk behind the whole aux batch, stalling the DVE pipeline ~4us)
        with tc.high_priority():
            emit_x2_bcast(0)

        mall = const.tile([P, Q], F32)    # min_j(2 e_i.e_j - x2_j)
        pv = paux.tile([P, Q * 20 + 24], F32)  # [S | d_cand] + pout column

        def win_lhs(t):
            if t < 4:
                return aA[:, t * P:(t + 1) * P]
            if t < TB:
                return aB[:, (t - 4) * P:(t - 3) * P]
            return aC[:, (t - TB) * P:(t - TB + 1) * P]

        def emit_aux(t):
            # deferred one tile behind the windows: results are only read by
            # the epilogue.  (Per-tile tile_wait_until staggering was tried
            # and REGRESSED: the model-time hints insert real scheduling
            # bubbles; the early aux blob costs less than the bubbles.)
            scol = slice(t * 20, (t + 1) * 20)
            nc.tensor.matmul(pv[:, scol], win_lhs(t), sc_sb[:, scol],
                             start=True, stop=False)
            nc.tensor.matmul(pv[:, scol], a2_sb[:, t * P:(t + 1) * P],
                             sc2_sb[:, scol], start=False, stop=True)

        for t in range(Q):
            blk = 0 if t < TB else 1
            lhs = win_lhs(t)
            rhs1 = aA[:, 0:W1] if blk == 0 else web[:, 0:W1]
            rhs2 = aB[:, 0:W2] if blk == 0 else web[:, W1:Wr]

            # the whole critical chain (descriptors above, x2 broadcasts,
            # windows) is pinned at priority 0 in consumption order; the aux
            # pairs fill the remaining tensor slots.  Without this the
            # scheduler models the custom DVE ops as slow, concludes the
            # windows are psum-blocked, and packs the whole aux blob ahead
            # of them (3.9us DVE stall).
            ps = pwin.tile([P, Wr], F32, tag="ps", name=f"ps{t}")
            with tc.high_priority():
                nc.tensor.matmul(ps[:, 0:W1], lhs, rhs1, start=True,
                                 stop=True)
                nc.tensor.matmul(ps[:, W1:Wr], lhs, rhs2, start=True,
                                 stop=True)

            dsc = scratch.tile([P, Wr], F32, tag="dsc")
            nc.vector._custom_dve(ADD_MIN_REDUCE, out=dsc[:],
                                  in0=ps[:], in1=x2jp[blk][:],
                                  s0=POS_INF, accum_out=mall[:, t:t + 1])

            if t == 3:
                with tc.high_priority():
                    emit_x2_bcast(1)  # block-1 x2, needed from tile TB
            if t >= 1:
                emit_aux(t - 1)
        emit_aux(Q - 1)

        # ---- batched mining epilogue (reads the aux PSUM directly) ----
        # tile_wait_until pushes these into the scheduler's far future so the
        # in-order DVE queue keeps the mining ops (emitted above) first; the
        # v2 trace showed the scheduler hoisting this chain ahead of them.
        sv3 = pv[:, 0:Q * 20].rearrange("p (q s) -> p q s", s=20)
        smax = const.tile([P, Q], F32)
        mask = const.tile([P, Q * 10], F32)
        mask3 = mask[:].rearrange("p (q s) -> p q s", s=10)
        prod = const.tile([P, Q * 10], F32)
        prod3 = prod[:].rearrange("p (q s) -> p q s", s=10)
        dneg = const.tile([P, Q], F32)
        with tc.tile_wait_until(0.030):
            nc.vector.reduce_max(smax[:], sv3[:, :, 0:10], axis=AX)
            smax_b = smax[:].rearrange("p (q one) -> p q one", one=1) \
                .to_broadcast((P, Q, 10))
            nc.vector.tensor_tensor(mask3, sv3[:, :, 0:10], smax_b,
                                    op=ALU.is_equal)
            nc.vector.tensor_tensor(prod3, mask3, sv3[:, :, 10:20],
                                    op=ALU.mult)
            nc.vector.reduce_sum(dneg[:], prod3, axis=AX)

            # xm already carries +margin (folded host-side, exact in f32).
            # NOTE: tensor_scalar with accum_out (TensorScalarPtrReduce)
            # hard-faults this runtime (NRT_EXEC_UNIT_UNRECOVERABLE) — keep
            # the relu and the row-sum as separate instructions.
            t1 = const.tile([P, Q], F32)
            nc.vector.tensor_sub(t1[:], xm_sb[:], mall[:])  # x2_i+m - min
            t2 = const.tile([P, Q], F32)
            nc.vector.tensor_sub(t2[:], t1[:], dneg[:])
            t3 = const.tile([P, Q], F32)
            nc.vector.tensor_scalar(t3[:], t2[:], 0.0, None, op0=ALU.max)
            lsum = const.tile([P, 1], F32)
            nc.vector.reduce_sum(lsum[:], t3[:], axis=AX)
            # partition-sum via a 1-column matmul so the output DMA is a
            # single 4-byte transfer
            nc.tensor.matmul(pv[0:1, Q * 20 + 20:Q * 20 + 21], lsum[:],
                             ones_f[:], start=True, stop=True)
            res_sb = const.tile([1, 1], F32)
            nc.vector.tensor_copy(res_sb[:], pv[0:1, Q * 20 + 20:Q * 20 + 21])
            nc.sync.dma_start(out_d[:], res_sb[:])

    nc.compile()
    return nc


_prog_cache: dict = {}


def kernel(embeddings: np.ndarray, labels: np.ndarray) -> np.ndarray:
    global last_results
    e = np.ascontiguousarray(np.asarray(embeddings), dtype=np.float32)
    lab = np.asarray(labels).astype(np.int64)
    N, D = e.shape
    assert D == P and N % N_CORES == 0

    # ---- host-side marshalling: class-sort, pad, per-class stats ----
    order = np.argsort(lab * N + np.arange(N))
    e = e[order]
    lab_s = lab[order]
    cnt = np.bincount(lab_s, minlength=C)
    assert len(cnt) == C and cnt[0] >= 10 and cnt[1] >= 10, cnt
    offs = np.zeros(C + 1, dtype=np.int64)
    offs[1:] = np.cumsum(cnt)

    # block width: per-class tile count, uniform across classes (all counts
    # land in the same 128-bucket for this regime).  C*TB tiles split over
    # the cores; slots past the real tiles are loss-squashed dummies.
    B = int(-(-int(cnt.max()) // P) * P)
    TB = B // P
    Q = -(-C * TB // N_CORES)
    L = Q - TB            # leftover slots per core
    n_left = C - N_CORES  # leftover blocks (2)
    cpl = N_CORES // n_left  # cores sharing one leftover block
    assert n_left == 2 and L * cpl >= TB

    x2 = np.einsum("nd,nd->n", e, e).astype(np.float32)
    NP_ = C * B
    ep = np.empty((NP_, D), np.float32)
    x2p = np.empty(NP_, np.float32)
    validp = np.zeros(NP_, np.float32)
    for k in range(C):
        m = int(cnt[k])
        blk = e[offs[k]:offs[k + 1]]
        ep[k * B:k * B + m] = blk
        ep[k * B + m:(k + 1) * B] = blk[0]
        x2p[k * B:k * B + m] = x2[offs[k]:offs[k + 1]]
        x2p[k * B + m:(k + 1) * B] = x2[offs[k]]
        validp[k * B:k * B + m] = 1.0
    # bf16-rounded x2_i, shared by the fp32 aux matmul and x2a1 so the x2_i
    # term cancels exactly in d_pos - d_neg
    x2p_bf32 = x2p.astype(ml_dtypes.bfloat16).astype(np.float32)

    E = np.stack([e[offs[k]:offs[k + 1]].sum(axis=0) for k in range(C)],
                 axis=1).astype(np.float32)          # [D, C]
    Ck = np.array([x2[offs[k]:offs[k + 1]].sum() for k in range(C)],
                  dtype=np.float32)                  # [C]
    candA = e[0:10]                                  # class-0 members
    candB = e[offs[1]:offs[1] + 10]                  # class-1 members
    x2A, x2B = x2[0:10], x2[offs[1]:offs[1] + 10]

    Wr = int(cnt.max())
    key = (B, Q, Wr)
    if key not in _prog_cache:
        _prog_cache[key] = _build_program(B, Q, Wr)
    nc = _prog_cache[key]

    in_maps = []
    for c in range(N_CORES):
        mb = c                      # main block
        eb = N_CORES + c // cpl     # leftover block index
        base_et = (c % cpl) * L     # first leftover tile within it
        row_segs = [np.arange(mb * B, (mb + 1) * B)]
        dummy_slot = []
        for j in range(L):
            et = base_et + j
            if et >= TB:            # dummy slot: replicate tile 0, squash
                et = 0
                dummy_slot.append(TB + j)
            row_segs.append(np.arange(eb * B + et * P, eb * B + (et + 1) * P))
        rows = np.concatenate(row_segs)
        tile_cls = [mb] * TB + [eb] * L

        anchT = ep[rows].T                          # [D, Q*128]
        a = (SQ2 * anchT).astype(ml_dtypes.bfloat16)
        web = (SQ2 * ep[eb * B:eb * B + Wr].T).astype(ml_dtypes.bfloat16)
        x2r = -np.concatenate([x2p[mb * B:mb * B + Wr],
                               x2p[eb * B:eb * B + Wr]])[None, :] \
            .astype(ml_dtypes.bfloat16)
        # aux lhsT rows [x2_i; 1; x2_i; 1] pair with hi/lo-split rhs rows so
        # every aux product is bf16-exact (bf16 alone cannot hold cnt_k / C_k)
        a2 = np.stack([x2p_bf32[rows], np.ones(Q * P, np.float32),
                       x2p_bf32[rows], np.ones(Q * P, np.float32)])
        sc = np.empty((D, Q * 20), np.float32)
        sc2 = np.empty((4, Q * 20), np.float32)
        cnt_f = cnt.astype(np.float32)
        cnt_hi = (cnt // 128 * 128).astype(np.float32)
        cnt_lo = cnt_f - cnt_hi
        Ck_hi = Ck.astype(ml_dtypes.bfloat16).astype(np.float32)
        Ck_lo = Ck - Ck_hi
        x2A_hi = x2A.astype(ml_dtypes.bfloat16).astype(np.float32)
        x2B_hi = x2B.astype(ml_dtypes.bfloat16).astype(np.float32)
        for t in range(Q):
            c0 = tile_cls[t] == 0
            cand = candB if c0 else candA
            x2c_hi = x2B_hi if c0 else x2A_hi
            x2c_lo = (x2B - x2B_hi) if c0 else (x2A - x2A_hi)
            sc[:, t * 20:t * 20 + 10] = -SQ2 * E
            sc[:, t * 20 + 10:t * 20 + 20] = -SQ2 * cand.T
            sc2[0, t * 20:t * 20 + 10] = cnt_hi
            sc2[1, t * 20:t * 20 + 10] = Ck_hi
            sc2[2, t * 20:t * 20 + 10] = cnt_lo
            sc2[3, t * 20:t * 20 + 10] = Ck_lo
            sc2[0, t * 20 + 10:t * 20 + 20] = 1.0
            sc2[1, t * 20 + 10:t * 20 + 20] = x2c_hi
            sc2[2, t * 20 + 10:t * 20 + 20] = 0.0
            sc2[3, t * 20 + 10:t * 20 + 20] = x2c_lo
        vmask = validp[rows].reshape(Q, P).T.copy()
        for s in dummy_slot:
            vmask[:, s] = 0.0
        # margin folded in host-side (exact in f32; the bf16-rounded x2 term
        # still cancels against the aux-matmul x2)
        x2a1 = np.where(vmask > 0.5,
                        x2p_bf32[rows].reshape(Q, P).T + MARGIN,
                        PAD_NEG).astype(np.float32).copy()

        in_maps.append({"aA": np.ascontiguousarray(a[:, 0:512]),
                        "aB": np.ascontiguousarray(a[:, 512:TB * P]),
                        "aC": np.ascontiguousarray(a[:, TB * P:Q * P]),
                        "web": web,
                        "x2r": x2r,
                        "sc": sc.astype(ml_dtypes.bfloat16),
                        "sc2": sc2.astype(ml_dtypes.bfloat16),
                        "a2": a2.astype(ml_dtypes.bfloat16),
                        "x2a1": x2a1})

    res = run_bass_kernel_spmd(nc, in_maps, list(range(N_CORES)), **_trace_opts)
    last_results = res
    total = np.float64(0.0)
    for c in range(N_CORES):
        total += res.results[c]["out"].astype(np.float64).sum()
    return np.asarray(total / N, dtype=np.float32)
